# revision 5
# baseline (speedup 1.0000x reference)
"""CBOW negative-sampling loss on 8 Trainium2 NeuronCores.

Problem:  loss = mean_b[ softplus(-clip(pos_b)) + sum_k softplus(clip(neg_bk)) ]
  with pos_b  = mean_w(T[tgt[b,w]]) . C[ctx[b]]
       neg_bk = mean_w(T[tgt[b,w]]) . C[neg[b,k]]
  T/C are [100000, 128] f32 embedding tables, B=16384, W=K=10.

Strategy: data-parallel over batch (2048 elems/core).  The dominant cost is
the 21 gathered 512B table rows per batch element (~22 MB/core of random HBM
reads) done on-device with InstDMAGatherAnt (one SWDGE instruction per ~2.5K
rows).  dma_gather indices are int16, so each core's referenced table rows
are compacted host-side (np.unique) into a per-core table of at most
20480/22528 rows — indices then fit int16 while the device still performs the
full random gather.  Compute: window-sum via strided TensorReduce, the 11
dot-product families via scalar_tensor_tensor accumulating straight into a
per-core score matrix, then one clip + Exp + Ln(+1) pass (softplus) at the
end.  Per-element softplus terms are DMA'd out; the host does the final mean.
"""

import numpy as np

VOCAB = 100000
D = 128
B = 16384
W = 10
K = 10
NCORES = 8
BC = B // NCORES          # 2048 batch elements per core
NT = BC // 128            # 16 tiles of 128 batch elements
CHUNK_TILES = 2           # tiles per gather chunk
NCHUNK = NT // CHUNK_TILES
NIT = CHUNK_TILES * 128 * W   # target/negative rows gathered per chunk
UT = BC * W               # compact target-table rows (upper bound, 20480)
UC = BC * (K + 1)         # compact context-table rows (upper bound, 22528)

_cache = {}


def _build_module():
    import concourse.bacc as bacc
    import concourse.mybir as mybir
    from concourse.tile import TileContext

    f32 = mybir.dt.float32
    i16 = mybir.dt.int16
    AX = mybir.AxisListType
    OP = mybir.AluOpType
    ACT = mybir.ActivationFunctionType

    nc = bacc.Bacc("TRN2", debug=False, target_bir_lowering=False,
                   num_devices=NCORES)

    tab_t = nc.dram_tensor("tab_t", [UT, D], f32, kind="ExternalInput").ap()
    tab_c = nc.dram_tensor("tab_c", [UC, D], f32, kind="ExternalInput").ap()
    idx_t = nc.dram_tensor("idx_t", [128, NCHUNK * NIT // 16], i16,
                           kind="ExternalInput").ap()
    idx_n = nc.dram_tensor("idx_n", [128, NCHUNK * NIT // 16], i16,
                           kind="ExternalInput").ap()
    idx_c = nc.dram_tensor("idx_c", [128, BC // 16], i16,
                           kind="ExternalInput").ap()
    out = nc.dram_tensor("loss_out", [128, NT * 11], f32,
                         kind="ExternalOutput").ap()

    with TileContext(nc) as tc:
        with tc.tile_pool(name="const", bufs=1) as constp, \
             tc.tile_pool(name="gather", bufs=3) as gpool, \
             tc.tile_pool(name="work", bufs=3) as wpool:
            neg10 = constp.tile([128, 1], f32)
            nc.vector.memset(neg10, -10.0)
            # warm the ACT function table (exp+ln share one set) off the
            # critical path so the end-of-kernel softplus doesn't pay it
            warm = constp.tile([128, 2], f32)
            nc.scalar.activation(out=warm[:, 0:1], in_=neg10, func=ACT.Exp)
            nc.scalar.activation(out=warm[:, 1:2], in_=neg10, func=ACT.Ln,
                                 bias=1.0)

            # un-clipped scores for every tile: col t*11+c (c=0 -> -pos)
            scores_all = constp.tile([128, NT * 11], f32)

            # all gather index lists: one DMA each
            tidx = constp.tile([128, NCHUNK * NIT // 16], i16)
            nc.sync.dma_start(out=tidx, in_=idx_t)
            nidx = constp.tile([128, NCHUNK * NIT // 16], i16)
            nc.sync.dma_start(out=nidx, in_=idx_n)
            cidx = constp.tile([128, BC // 16], i16)
            nc.sync.dma_start(out=cidx, in_=idx_c)

            # context rows for all 16 tiles in one gather
            ctxbuf = constp.tile([128, NT * D], f32)
            nc.gpsimd.dma_gather(
                ctxbuf.rearrange("p (s d) -> p s d", d=D),
                tab_c, cidx, BC, BC, D, single_packet=False)

            IC = NIT // 16  # idx columns per chunk
            for ch in range(NCHUNK):
                tgtbuf = gpool.tile([128, CHUNK_TILES * W * D], f32,
                                    tag="tgtbuf")
                nc.gpsimd.dma_gather(
                    tgtbuf.rearrange("p (s d) -> p s d", d=D),
                    tab_t, tidx[:, ch * IC:(ch + 1) * IC], NIT, NIT, D,
                    single_packet=False)
                negbuf = gpool.tile([128, CHUNK_TILES * K * D], f32,
                                    tag="negbuf")
                nc.gpsimd.dma_gather(
                    negbuf.rearrange("p (s d) -> p s d", d=D),
                    tab_c, nidx[:, ch * IC:(ch + 1) * IC], NIT, NIT, D,
                    single_packet=False)

                for j in range(CHUNK_TILES):
                    t = ch * CHUNK_TILES + j
                    # window sum over the 10 gathered target rows
                    trg = wpool.tile([128, D], f32, tag="trg")
                    tv = tgtbuf[:, j * W * D:(j + 1) * W * D] \
                        .rearrange("p (w d) -> p d w", d=D)
                    nc.vector.tensor_reduce(out=trg, in_=tv, axis=AX.X,
                                            op=OP.add)

                    # 11 batched dot products, accumulated over d
                    sstx = wpool.tile([128, D], f32, tag="sstx")
                    nc.vector.scalar_tensor_tensor(
                        out=sstx, in0=trg, scalar=-1.0 / W,
                        in1=ctxbuf[:, t * D:(t + 1) * D],
                        op0=OP.mult, op1=OP.mult,
                        accum_out=scores_all[:, t * 11:t * 11 + 1])
                    for k in range(K):
                        s = j * K + k
                        nc.vector.scalar_tensor_tensor(
                            out=sstx, in0=trg, scalar=1.0 / W,
                            in1=negbuf[:, s * D:(s + 1) * D],
                            op0=OP.mult, op1=OP.mult,
                            accum_out=scores_all[:, t * 11 + 1 + k:
                                                 t * 11 + 2 + k])

            # clip to [-10, 10], then softplus(x) = Ln(Exp(x) + 1)
            clipped = constp.tile([128, NT * 11], f32)
            nc.vector.scalar_tensor_tensor(
                out=clipped, in0=scores_all, scalar=10.0,
                in1=neg10.to_broadcast([128, NT * 11]),
                op0=OP.min, op1=OP.max)
            expb = constp.tile([128, NT * 11], f32)
            nc.scalar.activation(out=expb, in_=clipped, func=ACT.Exp)
            lnb = constp.tile([128, NT * 11], f32)
            nc.scalar.activation(out=lnb, in_=expb, func=ACT.Ln, bias=1.0)

            nc.sync.dma_start(out=out, in_=lnb)

    nc.compile()
    return nc


def _get_module():
    if "nc" not in _cache:
        _cache["nc"] = _build_module()
    return _cache["nc"]


def _pack16(idx_list):
    """int16 index list -> [128, N/16] layout read by the Q7 gather kernel
    (position i lives at [i%16, i//16]; replicated for the 8 Q7 cores)."""
    n = idx_list.shape[0]
    assert n % 16 == 0
    m = np.ascontiguousarray(idx_list.astype(np.int16).reshape(n // 16, 16).T)
    return np.tile(m, (8, 1))


def _prep_core(target_table, context_table, tgt_c, ctx_c, neg_c):
    """Build one core's input map: compacted tables + int16 gather lists."""
    # target table: rows referenced by this core's window indices
    uniq_t, inv_t = np.unique(tgt_c.ravel(), return_inverse=True)
    tabt = np.zeros((UT, D), np.float32)
    tabt[:uniq_t.shape[0]] = target_table[uniq_t]
    inv_t = inv_t.reshape(BC, W)

    # context table: rows referenced by ctx + negatives
    refs = np.concatenate([ctx_c.ravel(), neg_c.ravel()])
    uniq_c, inv_c = np.unique(refs, return_inverse=True)
    tabc = np.zeros((UC, D), np.float32)
    tabc[:uniq_c.shape[0]] = context_table[uniq_c]
    inv_ctx = inv_c[:BC]
    inv_neg = inv_c[BC:].reshape(BC, K)

    # gather order: position i -> sbuf (partition i%128, slot i//128);
    # we want row (b, w) at partition b%128, slot (b_sub//128)*W + w.
    def chunk_lists(inv):  # inv: [BC, W]
        cols = []
        for ch in range(NCHUNK):
            blk = inv[ch * CHUNK_TILES * 128:(ch + 1) * CHUNK_TILES * 128]
            L = blk.reshape(CHUNK_TILES, 128, W).transpose(0, 2, 1).ravel()
            cols.append(_pack16(L))
        return np.hstack(cols)

    return {
        "tab_t": tabt,
        "tab_c": tabc,
        "idx_t": chunk_lists(inv_t),
        "idx_n": chunk_lists(inv_neg),
        "idx_c": _pack16(inv_ctx),
    }


def kernel(target_table, context_table, context, target, negatives):
    from concourse.bass_utils import run_bass_kernel_spmd

    target_table = np.asarray(target_table, np.float32)
    context_table = np.asarray(context_table, np.float32)
    context = np.asarray(context, np.int64)
    target = np.asarray(target, np.int64)
    negatives = np.asarray(negatives, np.int64)

    nc = _get_module()

    in_maps = []
    for c in range(NCORES):
        sl = slice(c * BC, (c + 1) * BC)
        in_maps.append(_prep_core(target_table, context_table,
                                  target[sl], context[sl], negatives[sl]))

    res = run_bass_kernel_spmd(nc, in_maps, core_ids=list(range(NCORES)),
                               trace=False)

    total = 0.0
    for r in res.results:
        total += float(np.asarray(r["loss_out"], np.float64).sum())
    return np.float32(total / B)


# revision 7
# speedup vs baseline: 1.0138x; 1.0138x over previous
"""CBOW negative-sampling loss on 8 Trainium2 NeuronCores.

Problem:  loss = mean_b[ softplus(-clip(pos_b)) + sum_k softplus(clip(neg_bk)) ]
  with pos_b  = mean_w(T[tgt[b,w]]) . C[ctx[b]]
       neg_bk = mean_w(T[tgt[b,w]]) . C[neg[b,k]]
  T/C are [100000, 128] f32 embedding tables, B=16384, W=K=10.

Strategy: data-parallel over batch (2048 elems/core).  The dominant cost is
the 21 gathered 512B table rows per batch element (~22 MB/core of random HBM
reads) done on-device with InstDMAGatherAnt (one SWDGE instruction per ~2.5K
rows).  dma_gather indices are int16, so each core's referenced table rows
are compacted host-side (np.unique) into a per-core table of at most
20480/22528 rows — indices then fit int16 while the device still performs the
full random gather.  Compute: window-sum via strided TensorReduce, the 11
dot-product families via scalar_tensor_tensor accumulating straight into a
per-core score matrix, then one clip + Exp + Ln(+1) pass (softplus) at the
end.  Per-element softplus terms are DMA'd out; the host does the final mean.
"""

import numpy as np

VOCAB = 100000
D = 128
B = 16384
W = 10
K = 10
NCORES = 8
BC = B // NCORES          # 2048 batch elements per core
NT = BC // 128            # 16 tiles of 128 batch elements
CHUNK_TILES = 2           # tiles per gather chunk
NCHUNK = NT // CHUNK_TILES
NIT = CHUNK_TILES * 128 * W   # target/negative rows gathered per chunk
UT = BC * W               # compact target-table rows (upper bound, 20480)
UC = BC * (K + 1)         # compact context-table rows (upper bound, 22528)

_cache = {}


def _build_module():
    import concourse.bacc as bacc
    import concourse.mybir as mybir
    from concourse.tile import TileContext

    f32 = mybir.dt.float32
    i16 = mybir.dt.int16
    AX = mybir.AxisListType
    OP = mybir.AluOpType
    ACT = mybir.ActivationFunctionType

    # Both Exp and Ln live in the 'natural_log_exp_and_others' ACT table
    # set, but the table-load pass picks the first set containing each
    # function, which alternates two sets (4 reloads, ~5us).  Strip Exp/Ln
    # from every other set (canonical order preserved) so one load covers
    # both.
    if not getattr(bacc.get_activation_tables, "_patched_explng", False):
        _orig_tables = bacc.get_activation_tables

        def _tables_one_expln_set(arch):
            t = _orig_tables(arch)
            for name, funcs in t.items():
                if name != "natural_log_exp_and_others":
                    funcs.discard(ACT.Exp)
                    funcs.discard(ACT.Ln)
            return t

        _tables_one_expln_set._patched_explng = True
        bacc.get_activation_tables = _tables_one_expln_set

    nc = bacc.Bacc("TRN2", debug=False, target_bir_lowering=False,
                   num_devices=NCORES)

    tab_t = nc.dram_tensor("tab_t", [UT, D], f32, kind="ExternalInput").ap()
    tab_c = nc.dram_tensor("tab_c", [UC, D], f32, kind="ExternalInput").ap()
    idx_t = nc.dram_tensor("idx_t", [128, NCHUNK * NIT // 16], i16,
                           kind="ExternalInput").ap()
    idx_n = nc.dram_tensor("idx_n", [128, NCHUNK * NIT // 16], i16,
                           kind="ExternalInput").ap()
    idx_c = nc.dram_tensor("idx_c", [128, BC // 16], i16,
                           kind="ExternalInput").ap()
    out = nc.dram_tensor("loss_out", [128, NT * 11], f32,
                         kind="ExternalOutput").ap()

    with TileContext(nc) as tc:
        with tc.tile_pool(name="const", bufs=1) as constp, \
             tc.tile_pool(name="gather", bufs=3) as gpool, \
             tc.tile_pool(name="work", bufs=3) as wpool:
            neg10 = constp.tile([128, 1], f32)
            nc.vector.memset(neg10, -10.0)
            # warm the ACT function table (exp+ln share one set) off the
            # critical path so the end-of-kernel softplus doesn't pay it
            warm = constp.tile([128, 2], f32)
            nc.scalar.activation(out=warm[:, 0:1], in_=neg10, func=ACT.Exp)
            nc.scalar.activation(out=warm[:, 1:2], in_=neg10, func=ACT.Ln,
                                 bias=1.0)

            # un-clipped scores for every tile: col t*11+c (c=0 -> -pos)
            scores_all = constp.tile([128, NT * 11], f32)

            # all gather index lists: one DMA each
            tidx = constp.tile([128, NCHUNK * NIT // 16], i16)
            nc.sync.dma_start(out=tidx, in_=idx_t)
            nidx = constp.tile([128, NCHUNK * NIT // 16], i16)
            nc.sync.dma_start(out=nidx, in_=idx_n)
            cidx = constp.tile([128, BC // 16], i16)
            nc.sync.dma_start(out=cidx, in_=idx_c)

            IC = NIT // 16  # idx columns per chunk

            def issue_chunk_gathers(ch):
                tgtbuf = gpool.tile([128, CHUNK_TILES * W * D], f32,
                                    tag="tgtbuf")
                nc.gpsimd.dma_gather(
                    tgtbuf.rearrange("p (s d) -> p s d", d=D),
                    tab_t, tidx[:, ch * IC:(ch + 1) * IC], NIT, NIT, D,
                    single_packet=False)
                negbuf = gpool.tile([128, CHUNK_TILES * K * D], f32,
                                    tag="negbuf")
                nc.gpsimd.dma_gather(
                    negbuf.rearrange("p (s d) -> p s d", d=D),
                    tab_c, nidx[:, ch * IC:(ch + 1) * IC], NIT, NIT, D,
                    single_packet=False)
                return tgtbuf, negbuf

            # chunk-0 gathers first so DVE ramps in as early as possible;
            # the ctx rows (one gather for all 16 tiles) are only needed by
            # the dot-product stage
            chunk0 = issue_chunk_gathers(0)
            ctxbuf = constp.tile([128, NT * D], f32)
            nc.gpsimd.dma_gather(
                ctxbuf.rearrange("p (s d) -> p s d", d=D),
                tab_c, cidx, BC, BC, D, single_packet=False)

            for ch in range(NCHUNK):
                tgtbuf, negbuf = chunk0 if ch == 0 else issue_chunk_gathers(ch)

                for j in range(CHUNK_TILES):
                    t = ch * CHUNK_TILES + j
                    # window sum over the 10 gathered target rows
                    trg = wpool.tile([128, D], f32, tag="trg")
                    tv = tgtbuf[:, j * W * D:(j + 1) * W * D] \
                        .rearrange("p (w d) -> p d w", d=D)
                    nc.vector.tensor_reduce(out=trg, in_=tv, axis=AX.X,
                                            op=OP.add)

                    # 11 batched dot products, accumulated over d
                    sstx = wpool.tile([128, D], f32, tag="sstx")
                    nc.vector.scalar_tensor_tensor(
                        out=sstx, in0=trg, scalar=-1.0 / W,
                        in1=ctxbuf[:, t * D:(t + 1) * D],
                        op0=OP.mult, op1=OP.mult,
                        accum_out=scores_all[:, t * 11:t * 11 + 1])
                    for k in range(K):
                        s = j * K + k
                        nc.vector.scalar_tensor_tensor(
                            out=sstx, in0=trg, scalar=1.0 / W,
                            in1=negbuf[:, s * D:(s + 1) * D],
                            op0=OP.mult, op1=OP.mult,
                            accum_out=scores_all[:, t * 11 + 1 + k:
                                                 t * 11 + 2 + k])

            # clip to [-10, 10], then softplus(x) = Ln(Exp(x) + 1)
            clipped = constp.tile([128, NT * 11], f32)
            nc.vector.scalar_tensor_tensor(
                out=clipped, in0=scores_all, scalar=10.0,
                in1=neg10.to_broadcast([128, NT * 11]),
                op0=OP.min, op1=OP.max)
            expb = constp.tile([128, NT * 11], f32)
            nc.scalar.activation(out=expb, in_=clipped, func=ACT.Exp)
            lnb = constp.tile([128, NT * 11], f32)
            nc.scalar.activation(out=lnb, in_=expb, func=ACT.Ln, bias=1.0)

            nc.sync.dma_start(out=out, in_=lnb)

    nc.compile()
    return nc


def _get_module():
    if "nc" not in _cache:
        _cache["nc"] = _build_module()
    return _cache["nc"]


def _pack16(idx_list):
    """int16 index list -> [128, N/16] layout read by the Q7 gather kernel
    (position i lives at [i%16, i//16]; replicated for the 8 Q7 cores)."""
    n = idx_list.shape[0]
    assert n % 16 == 0
    m = np.ascontiguousarray(idx_list.astype(np.int16).reshape(n // 16, 16).T)
    return np.tile(m, (8, 1))


def _prep_core(target_table, context_table, tgt_c, ctx_c, neg_c):
    """Build one core's input map: compacted tables + int16 gather lists."""
    # target table: rows referenced by this core's window indices
    uniq_t, inv_t = np.unique(tgt_c.ravel(), return_inverse=True)
    tabt = np.zeros((UT, D), np.float32)
    tabt[:uniq_t.shape[0]] = target_table[uniq_t]
    inv_t = inv_t.reshape(BC, W)

    # context table: rows referenced by ctx + negatives
    refs = np.concatenate([ctx_c.ravel(), neg_c.ravel()])
    uniq_c, inv_c = np.unique(refs, return_inverse=True)
    tabc = np.zeros((UC, D), np.float32)
    tabc[:uniq_c.shape[0]] = context_table[uniq_c]
    inv_ctx = inv_c[:BC]
    inv_neg = inv_c[BC:].reshape(BC, K)

    # gather order: position i -> sbuf (partition i%128, slot i//128);
    # we want row (b, w) at partition b%128, slot (b_sub//128)*W + w.
    def chunk_lists(inv):  # inv: [BC, W]
        cols = []
        for ch in range(NCHUNK):
            blk = inv[ch * CHUNK_TILES * 128:(ch + 1) * CHUNK_TILES * 128]
            L = blk.reshape(CHUNK_TILES, 128, W).transpose(0, 2, 1).ravel()
            cols.append(_pack16(L))
        return np.hstack(cols)

    return {
        "tab_t": tabt,
        "tab_c": tabc,
        "idx_t": chunk_lists(inv_t),
        "idx_n": chunk_lists(inv_neg),
        "idx_c": _pack16(inv_ctx),
    }


def kernel(target_table, context_table, context, target, negatives):
    from concourse.bass_utils import run_bass_kernel_spmd

    target_table = np.asarray(target_table, np.float32)
    context_table = np.asarray(context_table, np.float32)
    context = np.asarray(context, np.int64)
    target = np.asarray(target, np.int64)
    negatives = np.asarray(negatives, np.int64)

    nc = _get_module()

    in_maps = []
    for c in range(NCORES):
        sl = slice(c * BC, (c + 1) * BC)
        in_maps.append(_prep_core(target_table, context_table,
                                  target[sl], context[sl], negatives[sl]))

    res = run_bass_kernel_spmd(nc, in_maps, core_ids=list(range(NCORES)),
                               trace=False)

    total = 0.0
    for r in res.results:
        total += float(np.asarray(r["loss_out"], np.float64).sum())
    return np.float32(total / B)
